# revision 30
# baseline (speedup 1.0000x reference)
"""GuidedFilter (3-angle iterated boxfilter) on 8 trn2 NeuronCores.

Math: reference iterates  X <- X + (B_a(y) - B_a(X))/N_a  over 3 rotated-line
kernels B_a.  With the residual D = y - X this is  D <- D - B_a(D)/N_a,
one conv per angle, and  X_final = y - D_final  (computed on host).

Mapping: core (b, h) = (i//4, i%4) handles batch b, rows [512h, 512h+512).
Each core gets a 576-row slab (24-row shrink-halo per side, no cross-core
traffic), processed as 5 overlapping 128-row chunks (stride 112).  Per angle
and chunk the whole update Dnew = D - g_row*B'(D) (B' = B_a/s_a, identity
folded into the center-column band) is built on the TensorEngine as one
banded [128,128] bf16 matmul per nonzero kernel column (5 for +-10deg,
1 for 0deg), accumulated in PSUM.  ScalarE copies PSUM->SBUF bf16 per
512-column tile; DVE re-derives the 4 edge columns (where the N normalizer
differs per column) from src/dst:  Dnew = src - gc*(src - dst_wrong).
Chunk-overlap rows are synced by SBUF-to-SBUF DMA between passes.  D_final
goes out bf16; the host computes X = y - D in f32.  Dummy matmuls at start
keep the PE HAM clock-gate warm through the input-DMA phase.
"""

import numpy as np
import ml_dtypes

M_IMG = 2048
N_IMG = 2048
BATCH = 2
H_SHARDS = 4
SH = 512
SLAB = 576          # 512 + 2*32 shrink halo
CW = 2052           # bf16 master tile width (2 zero-pad cols each side)
NCHUNK = 5
CH_STEP = 112
KH = 17
PC = 8
PINGW = NCHUNK * CW

COLS = [(0, 1, 2, 3, 4), (2,), (0, 1, 2, 3, 4)]   # nonzero dx per angle
OFFS = [0, 15, 18]                                 # weight block base per angle
NWB = 33
OUT_ROWS = [(0, 32, 120), (88, 8, 120), (200, 8, 120), (312, 8, 120), (424, 8, 96)]
N_WARM = 30


def _host_prep(X, y, kern, N_norm):
    kern = np.asarray(kern, np.float64)[:, 0]        # (3,17,5)
    N = np.asarray(N_norm, np.float64)[:, 0]         # (3,2048,2048)
    D0 = (np.asarray(y) - np.asarray(X))[:, 0]       # (2,2048,2048) f32

    s = kern.sum(axis=(1, 2))
    grow_full = np.ones((3, M_IMG), np.float64)
    for a in range(3):
        grow_full[a] = s[a] / N[a, :, N_IMG // 2]

    BF16 = ml_dtypes.bfloat16

    in_maps = []
    for core in range(BATCH * H_SHARDS):
        b, h = core // H_SHARDS, core % H_SHARDS
        gs = SH * h - 32

        d0s = np.zeros((SLAB, CW), np.float32)
        r0, r1 = max(0, gs), min(M_IMG, gs + SLAB)
        d0s[r0 - gs:r1 - gs, 2:2 + N_IMG] = D0[b, r0:r1]

        # banded update matrices W = I - g*band, layout [k, idx*128 + m]
        wb = np.zeros((128, NWB * 128), np.float64)
        for a in range(3):
            for v in range(3):
                cv = {0: 0, 1: 1, 2: 4}[v]
                for m in range(8, 120):
                    go = gs + CH_STEP * cv + m
                    if not (0 <= go < M_IMG):
                        continue
                    g = grow_full[a][go]
                    for i, dx in enumerate(COLS[a]):
                        idx = OFFS[a] + v * len(COLS[a]) + i
                        for dy in range(KH):
                            if kern[a, dy, dx] == 0.0:
                                continue
                            wb[m - PC + dy, idx * 128 + m] -= \
                                g * kern[a, dy, dx] / s[a]
                        if dx == 2:
                            wb[m, idx * 128 + m] += 1.0

        # per-column strip factors gc = N(r,center)/N(r,c)
        gcs = np.ones((128, 3 * NCHUNK * 4), np.float64)
        scol = [0, 1, N_IMG - 2, N_IMG - 1]
        for a in range(3):
            for c in range(NCHUNK):
                g_glob = gs + CH_STEP * c + np.arange(128)
                ok = (g_glob >= 0) & (g_glob < M_IMG)
                gg = np.clip(g_glob, 0, M_IMG - 1)
                for j, cc in enumerate(scol):
                    v = N[a, gg, N_IMG // 2] / N[a, gg, cc]
                    gcs[:, a * 20 + c * 4 + j] = np.where(ok, v, 1.0)

        in_maps.append({
            "d0": d0s.astype(BF16),
            "wb": wb.astype(BF16),
            "gcs": gcs.astype(np.float32),
        })
    return in_maps


def _build_program():
    import concourse.bass as bass
    from concourse import mybir
    from contextlib import ExitStack

    f32 = mybir.dt.float32
    bf16 = mybir.dt.bfloat16
    nc = bass.Bass("TRN2", target_bir_lowering=False)

    d0 = nc.dram_tensor("d0", [SLAB, CW], bf16, kind="ExternalInput")
    wbd = nc.dram_tensor("wb", [128, NWB * 128], bf16, kind="ExternalInput")
    gcsd = nc.dram_tensor("gcs", [128, 60], f32, kind="ExternalInput")
    xo = nc.dram_tensor("xo", [SH, N_IMG], bf16, kind="ExternalOutput")

    ping = nc.alloc_sbuf_tensor("ping", [128, PINGW], bf16)
    pong = nc.alloc_sbuf_tensor("pong", [128, PINGW], bf16)
    wbs = nc.alloc_sbuf_tensor("wbs", [128, NWB * 128], bf16)
    gcsb = nc.alloc_sbuf_tensor("gcsb", [128, 60], f32)
    wm = nc.alloc_sbuf_tensor("wm", [128, 128], bf16)
    t1 = nc.alloc_sbuf_tensor("t1", [128, 8], f32)
    t2 = nc.alloc_sbuf_tensor("t2", [128, 8], f32)
    ps = [nc.alloc_psum_tensor(f"ps{i}", [128, N_IMG], f32) for i in range(2)]

    SRC = [ping, pong, ping]
    DST = [pong, ping, pong]

    def strip_ap(t, base, w):
        return bass.AP(t, base, [[w, 128], [2046, 2], [1, 2]])

    stack = ExitStack()
    with nc.Block() as block, \
         nc.semaphore("s_pe") as s_pe, nc.semaphore("s_cpy") as s_cpy, \
         nc.semaphore("s_str") as s_str, nc.semaphore("s_wa0") as s_wa0, \
         nc.semaphore("s_w2") as s_w2, nc.semaphore("s_out") as s_out, \
         nc.semaphore("s_pad") as s_pad, nc.semaphore("s_t1") as s_t1, \
         nc.semaphore("s_d34") as s_d34, nc.semaphore("s_cp2") as s_cp2, \
         nc.semaphore("s_wav0") as s_wav0:
        s_d0 = [stack.enter_context(nc.semaphore(f"s_d{c}")) for c in range(3)]
        s_h = [stack.enter_context(nc.semaphore(f"s_h{i}")) for i in range(8)]

        @block.sync
        def _(sp):
            sp.dma_start(out=wbs[:, 0:5 * 128],
                         in_=wbd[:, 0:5 * 128]).then_inc(s_wav0, 16)
            sp.dma_start(out=ping[:, 0:CW],
                         in_=d0[0:128, :]).then_inc(s_d0[0], 16)
            sp.dma_start(out=wbs[:, 5 * 128:OFFS[1] * 128],
                         in_=wbd[:, 5 * 128:OFFS[1] * 128]).then_inc(s_wa0, 16)
            sp.dma_start(out=gcsb[:, :], in_=gcsd[:, :]).then_inc(s_w2, 16)
            sp.dma_start(out=ping[:, CW:2 * CW],
                         in_=d0[CH_STEP:CH_STEP + 128, :]).then_inc(s_d0[1], 16)
            sp.dma_start(out=wbs[:, OFFS[1] * 128:NWB * 128],
                         in_=wbd[:, OFFS[1] * 128:NWB * 128]
                         ).then_inc(s_w2, 16)
            sp.dma_start(out=ping[:, 2 * CW:3 * CW],
                         in_=d0[2 * CH_STEP:2 * CH_STEP + 128, :]
                         ).then_inc(s_d0[2], 16)
            for c in (3, 4):
                sp.dma_start(out=ping[:, c * CW:(c + 1) * CW],
                             in_=d0[c * CH_STEP:c * CH_STEP + 128, :]
                             ).then_inc(s_d34, 16)
            # halo exchanges after pass 0 (on pong) and pass 1 (on ping)
            sp.wait_ge(s_pad, NCHUNK + 1)
            for t, tile in enumerate((pong, ping)):
                for bdy in range(NCHUNK - 1):
                    gbase = 5 * t + bdy
                    sp.wait_ge(s_cpy, 2 * (gbase + 2))
                    sp.wait_ge(s_cp2, 2 * (gbase + 2))
                    sp.wait_ge(s_str, gbase + 2)
                    sp.dma_start(
                        out=tile[0:8, (bdy + 1) * CW:(bdy + 2) * CW],
                        in_=tile[112:120, bdy * CW:(bdy + 1) * CW]
                    ).then_inc(s_h[4 * t + bdy], 16)
                    sp.dma_start(
                        out=tile[120:128, bdy * CW:(bdy + 1) * CW],
                        in_=tile[8:16, (bdy + 1) * CW:(bdy + 2) * CW]
                    ).then_inc(s_h[4 * t + bdy], 16)
            for c in range(NCHUNK):
                o, p0, p1 = OUT_ROWS[c]
                sp.wait_ge(s_cpy, 2 * (11 + c))
                sp.wait_ge(s_cp2, 2 * (11 + c))
                sp.wait_ge(s_str, 11 + c)
                sp.dma_start(out=xo[o:o + (p1 - p0), :],
                             in_=pong[p0:p1, c * CW + 2:c * CW + 2 + N_IMG]
                             ).then_inc(s_out, 16)
            sp.wait_ge(s_out, 16 * NCHUNK)

        @block.tensor
        def _(pe):
            pe.wait_ge(s_pad, 1)
            for i in range(N_WARM):
                pe.matmul(ps[1][0:64, 0:64], lhsT=wm[:, 0:64],
                          rhs=wm[:, 64:128], start=True, stop=True)
            for p in range(3):
                dxs = COLS[p]
                for c in range(NCHUNK):
                    g = 5 * p + c
                    v = {0: 0, 4: 2}.get(c, 1)
                    if g >= 2:
                        pe.wait_ge(s_cpy, 2 * (g - 1))
                        pe.wait_ge(s_cp2, 2 * (g - 1))
                    if p == 0:
                        pe.wait_ge(s_wav0, 16)
                        if c > 0:
                            pe.wait_ge(s_wa0, 16)
                        if c < 3:
                            pe.wait_ge(s_d0[c], 16)
                        else:
                            pe.wait_ge(s_d34, 32)
                    else:
                        pe.wait_ge(s_w2, 32)
                        hb = 4 * (p - 1)
                        if c > 0:
                            pe.wait_ge(s_h[hb + c - 1], 32)
                        pe.wait_ge(s_h[hb + min(c, 3)], 32)
                    pst = ps[g % 2]
                    for nt in range(4):
                        for i, dx in enumerate(dxs):
                            idx = OFFS[p] + v * len(dxs) + i
                            mm = pe.matmul(
                                pst[:, nt * 512:(nt + 1) * 512],
                                lhsT=wbs[:, idx * 128:(idx + 1) * 128],
                                rhs=SRC[p][:, c * CW + dx + nt * 512:
                                           c * CW + dx + nt * 512 + 512],
                                start=(i == 0), stop=(i == len(dxs) - 1))
                            if i == len(dxs) - 1:
                                mm.then_inc(s_pe, 1)

        @block.scalar
        def _(act):
            for g in range(15):
                p, c = divmod(g, 5)
                for nt in range(2):
                    act.wait_ge(s_pe, 4 * g + nt + 1)
                    act.copy(
                        out=DST[p][:, c * CW + 2 + nt * 512:
                                   c * CW + 2 + (nt + 1) * 512],
                        in_=ps[g % 2][:, nt * 512:(nt + 1) * 512]
                    ).then_inc(s_cpy, 1)

        @block.vector
        def _(dve):
            dve.memset(wm[:, :], 0.0).then_inc(s_pad, 1)
            for c in range(NCHUNK):
                dve.memset(bass.AP(pong, c * CW,
                                   [[PINGW, 128], [2050, 2], [1, 2]]),
                           0.0).then_inc(s_pad, 1)
            dve.wait_ge(s_w2, 32)
            for g in range(15):
                p, c = divmod(g, 5)
                par = 4 * (g % 2)
                t1_ap = bass.AP(t1, par, [[8, 128], [2, 2], [1, 2]])
                t2_ap = bass.AP(t2, par, [[8, 128], [2, 2], [1, 2]])
                gc_ap = bass.AP(gcsb, p * 20 + c * 4,
                                [[60, 128], [2, 2], [1, 2]])
                sstrip = strip_ap(SRC[p], c * CW + 2, PINGW)
                dstrip = strip_ap(DST[p], c * CW + 2, PINGW)
                for nt in (2, 3):
                    dve.wait_ge(s_pe, 4 * g + nt + 1)
                    dve.tensor_copy(
                        DST[p][:, c * CW + 2 + nt * 512:
                               c * CW + 2 + (nt + 1) * 512],
                        ps[g % 2][:, nt * 512:(nt + 1) * 512]
                    ).then_inc(s_cp2, 1)
                dve.wait_ge(s_cpy, 2 * (g + 1))
                dve.wait_ge(s_cp2, 2 * (g + 1))
                if g >= 2:
                    dve.wait_ge(s_str, g - 1)
                dve.tensor_sub(t1_ap, sstrip, dstrip).then_inc(s_t1, 1)
                dve.wait_ge(s_t1, 2 * g + 1)
                dve.tensor_mul(t2_ap, gc_ap, t1_ap).then_inc(s_t1, 1)
                dve.wait_ge(s_t1, 2 * g + 2)
                dve.tensor_sub(dstrip, sstrip, t2_ap).then_inc(s_str, 1)
    return nc


_LAST = None  # BassKernelResults of the most recent run (for test harness)


def kernel(X, y, kernel, N_norm):
    global _LAST
    from concourse.bass_utils import run_bass_kernel_spmd

    in_maps = _host_prep(X, y, kernel, N_norm)
    nc = _build_program()
    res = run_bass_kernel_spmd(nc, in_maps, list(range(BATCH * H_SHARDS)))
    _LAST = res

    yf = np.asarray(y)
    out = np.empty((BATCH, 1, M_IMG, N_IMG), np.float32)
    for core in range(BATCH * H_SHARDS):
        b, h = core // H_SHARDS, core % H_SHARDS
        out[b, 0, SH * h:SH * h + SH, :] = (
            yf[b, 0, SH * h:SH * h + SH, :]
            - res.results[core]["xo"].astype(np.float32))
    return out


# revision 31
# speedup vs baseline: 1.0049x; 1.0049x over previous
"""GuidedFilter (3-angle iterated boxfilter) on 8 trn2 NeuronCores.

Math: reference iterates  X <- X + (B_a(y) - B_a(X))/N_a  over 3 rotated-line
kernels B_a.  With the residual D = y - X this is  D <- D - B_a(D)/N_a,
one conv per angle, and  X_final = y - D_final  (computed on host).

Mapping: core (b, h) = (i//4, i%4) handles batch b, rows [512h, 512h+512).
Each core gets a 576-row slab (24-row shrink-halo per side, no cross-core
traffic), processed as 5 overlapping 128-row chunks (stride 112).  Per angle
and chunk the whole update Dnew = D - g_row*B'(D) (B' = B_a/s_a, identity
folded into the center-column band) is built on the TensorEngine as one
banded [128,128] bf16 matmul per nonzero kernel column (5 for +-10deg,
1 for 0deg), accumulated in PSUM.  ScalarE copies PSUM->SBUF bf16 per
512-column tile; DVE re-derives the 4 edge columns (where the N normalizer
differs per column) from src/dst:  Dnew = src - gc*(src - dst_wrong).
Chunk-overlap rows are synced by SBUF-to-SBUF DMA between passes.  D_final
goes out bf16; the host computes X = y - D in f32.  Dummy matmuls at start
keep the PE HAM clock-gate warm through the input-DMA phase.
"""

import numpy as np
import ml_dtypes

M_IMG = 2048
N_IMG = 2048
BATCH = 2
H_SHARDS = 4
SH = 512
SLAB = 576          # 512 + 2*32 shrink halo
CW = 2052           # bf16 master tile width (2 zero-pad cols each side)
NCHUNK = 5
CH_STEP = 112
KH = 17
PC = 8
PINGW = NCHUNK * CW

COLS = [(0, 1, 2, 3, 4), (2,), (0, 1, 2, 3, 4)]   # nonzero dx per angle
OFFS = [0, 15, 18]                                 # weight block base per angle
NWB = 33
OUT_ROWS = [(0, 32, 120), (88, 8, 120), (200, 8, 120), (312, 8, 120), (424, 8, 96)]
N_WARM = 40


def _host_prep(X, y, kern, N_norm):
    kern = np.asarray(kern, np.float64)[:, 0]        # (3,17,5)
    N = np.asarray(N_norm, np.float64)[:, 0]         # (3,2048,2048)
    D0 = (np.asarray(y) - np.asarray(X))[:, 0]       # (2,2048,2048) f32

    s = kern.sum(axis=(1, 2))
    grow_full = np.ones((3, M_IMG), np.float64)
    for a in range(3):
        grow_full[a] = s[a] / N[a, :, N_IMG // 2]

    BF16 = ml_dtypes.bfloat16

    in_maps = []
    for core in range(BATCH * H_SHARDS):
        b, h = core // H_SHARDS, core % H_SHARDS
        gs = SH * h - 32

        d0s = np.zeros((SLAB, CW), np.float32)
        r0, r1 = max(0, gs), min(M_IMG, gs + SLAB)
        d0s[r0 - gs:r1 - gs, 2:2 + N_IMG] = D0[b, r0:r1]

        # banded update matrices W = I - g*band, layout [k, idx*128 + m]
        wb = np.zeros((128, NWB * 128), np.float64)
        for a in range(3):
            for v in range(3):
                cv = {0: 0, 1: 1, 2: 4}[v]
                for m in range(8, 120):
                    go = gs + CH_STEP * cv + m
                    if not (0 <= go < M_IMG):
                        continue
                    g = grow_full[a][go]
                    for i, dx in enumerate(COLS[a]):
                        idx = OFFS[a] + v * len(COLS[a]) + i
                        for dy in range(KH):
                            if kern[a, dy, dx] == 0.0:
                                continue
                            wb[m - PC + dy, idx * 128 + m] -= \
                                g * kern[a, dy, dx] / s[a]
                        if dx == 2:
                            wb[m, idx * 128 + m] += 1.0

        # per-column strip factors gc = N(r,center)/N(r,c)
        gcs = np.ones((128, 3 * NCHUNK * 4), np.float64)
        scol = [0, 1, N_IMG - 2, N_IMG - 1]
        for a in range(3):
            for c in range(NCHUNK):
                g_glob = gs + CH_STEP * c + np.arange(128)
                ok = (g_glob >= 0) & (g_glob < M_IMG)
                gg = np.clip(g_glob, 0, M_IMG - 1)
                for j, cc in enumerate(scol):
                    v = N[a, gg, N_IMG // 2] / N[a, gg, cc]
                    gcs[:, a * 20 + c * 4 + j] = np.where(ok, v, 1.0)

        in_maps.append({
            "d0": d0s.astype(BF16),
            "wb": wb.astype(BF16),
            "gcs": gcs.astype(np.float32),
        })
    return in_maps


def _build_program():
    import concourse.bass as bass
    from concourse import mybir
    from contextlib import ExitStack

    f32 = mybir.dt.float32
    bf16 = mybir.dt.bfloat16
    nc = bass.Bass("TRN2", target_bir_lowering=False)

    d0 = nc.dram_tensor("d0", [SLAB, CW], bf16, kind="ExternalInput")
    wbd = nc.dram_tensor("wb", [128, NWB * 128], bf16, kind="ExternalInput")
    gcsd = nc.dram_tensor("gcs", [128, 60], f32, kind="ExternalInput")
    xo = nc.dram_tensor("xo", [SH, N_IMG], bf16, kind="ExternalOutput")

    ping = nc.alloc_sbuf_tensor("ping", [128, PINGW], bf16)
    pong = nc.alloc_sbuf_tensor("pong", [128, PINGW], bf16)
    wbs = nc.alloc_sbuf_tensor("wbs", [128, NWB * 128], bf16)
    gcsb = nc.alloc_sbuf_tensor("gcsb", [128, 60], f32)
    wm = nc.alloc_sbuf_tensor("wm", [128, 128], bf16)
    t1 = nc.alloc_sbuf_tensor("t1", [128, 8], f32)
    t2 = nc.alloc_sbuf_tensor("t2", [128, 8], f32)
    ps = [nc.alloc_psum_tensor(f"ps{i}", [128, N_IMG], f32) for i in range(2)]

    SRC = [ping, pong, ping]
    DST = [pong, ping, pong]

    def strip_ap(t, base, w):
        return bass.AP(t, base, [[w, 128], [2046, 2], [1, 2]])

    stack = ExitStack()
    with nc.Block() as block, \
         nc.semaphore("s_pe") as s_pe, nc.semaphore("s_cpy") as s_cpy, \
         nc.semaphore("s_str") as s_str, nc.semaphore("s_wa0") as s_wa0, \
         nc.semaphore("s_w2") as s_w2, nc.semaphore("s_out") as s_out, \
         nc.semaphore("s_pad") as s_pad, nc.semaphore("s_t1") as s_t1, \
         nc.semaphore("s_d34") as s_d34, nc.semaphore("s_cp2") as s_cp2, \
         nc.semaphore("s_wav0") as s_wav0:
        s_d0 = [stack.enter_context(nc.semaphore(f"s_d{c}")) for c in range(3)]
        s_h = [stack.enter_context(nc.semaphore(f"s_h{i}")) for i in range(8)]

        @block.sync
        def _(sp):
            sp.dma_start(out=ping[:, 0:CW],
                         in_=d0[0:128, :]).then_inc(s_d0[0], 16)
            sp.dma_start(out=wbs[:, 0:5 * 128],
                         in_=wbd[:, 0:5 * 128]).then_inc(s_wav0, 16)
            sp.dma_start(out=wbs[:, 5 * 128:OFFS[1] * 128],
                         in_=wbd[:, 5 * 128:OFFS[1] * 128]).then_inc(s_wa0, 16)
            sp.dma_start(out=gcsb[:, :], in_=gcsd[:, :]).then_inc(s_w2, 16)
            sp.dma_start(out=ping[:, CW:2 * CW],
                         in_=d0[CH_STEP:CH_STEP + 128, :]).then_inc(s_d0[1], 16)
            sp.dma_start(out=wbs[:, OFFS[1] * 128:NWB * 128],
                         in_=wbd[:, OFFS[1] * 128:NWB * 128]
                         ).then_inc(s_w2, 16)
            sp.dma_start(out=ping[:, 2 * CW:3 * CW],
                         in_=d0[2 * CH_STEP:2 * CH_STEP + 128, :]
                         ).then_inc(s_d0[2], 16)
            for c in (3, 4):
                sp.dma_start(out=ping[:, c * CW:(c + 1) * CW],
                             in_=d0[c * CH_STEP:c * CH_STEP + 128, :]
                             ).then_inc(s_d34, 16)
            # halo exchanges after pass 0 (on pong) and pass 1 (on ping)
            sp.wait_ge(s_pad, NCHUNK + 1)
            for t, tile in enumerate((pong, ping)):
                for bdy in range(NCHUNK - 1):
                    gbase = 5 * t + bdy
                    sp.wait_ge(s_cpy, gbase + 2)
                    sp.wait_ge(s_cp2, gbase + 2)
                    sp.wait_ge(s_str, gbase + 2)
                    sp.dma_start(
                        out=tile[0:8, (bdy + 1) * CW:(bdy + 2) * CW],
                        in_=tile[112:120, bdy * CW:(bdy + 1) * CW]
                    ).then_inc(s_h[4 * t + bdy], 16)
                    sp.dma_start(
                        out=tile[120:128, bdy * CW:(bdy + 1) * CW],
                        in_=tile[8:16, (bdy + 1) * CW:(bdy + 2) * CW]
                    ).then_inc(s_h[4 * t + bdy], 16)
            for c in range(NCHUNK):
                o, p0, p1 = OUT_ROWS[c]
                sp.wait_ge(s_cpy, 11 + c)
                sp.wait_ge(s_cp2, 11 + c)
                sp.wait_ge(s_str, 11 + c)
                sp.dma_start(out=xo[o:o + (p1 - p0), :],
                             in_=pong[p0:p1, c * CW + 2:c * CW + 2 + N_IMG]
                             ).then_inc(s_out, 16)
            sp.wait_ge(s_out, 16 * NCHUNK)

        @block.tensor
        def _(pe):
            pe.wait_ge(s_pad, 1)
            for i in range(N_WARM):
                pe.matmul(ps[1][0:64, 0:64], lhsT=wm[:, 0:64],
                          rhs=wm[:, 64:128], start=True, stop=True)
            for p in range(3):
                dxs = COLS[p]
                for c in range(NCHUNK):
                    g = 5 * p + c
                    v = {0: 0, 4: 2}.get(c, 1)
                    if g >= 2:
                        pe.wait_ge(s_cpy, g - 1)
                    if p == 0:
                        pe.wait_ge(s_wav0, 16)
                        if c > 0:
                            pe.wait_ge(s_wa0, 16)
                        if c < 3:
                            pe.wait_ge(s_d0[c], 16)
                        else:
                            pe.wait_ge(s_d34, 32)
                    else:
                        pe.wait_ge(s_w2, 32)
                        hb = 4 * (p - 1)
                        if c > 0:
                            pe.wait_ge(s_h[hb + c - 1], 32)
                        pe.wait_ge(s_h[hb + min(c, 3)], 32)
                    pst = ps[g % 2]
                    for nt in range(4):
                        if nt == 2 and g >= 2:
                            pe.wait_ge(s_cp2, g - 1)
                        for i, dx in enumerate(dxs):
                            idx = OFFS[p] + v * len(dxs) + i
                            mm = pe.matmul(
                                pst[:, nt * 512:(nt + 1) * 512],
                                lhsT=wbs[:, idx * 128:(idx + 1) * 128],
                                rhs=SRC[p][:, c * CW + dx + nt * 512:
                                           c * CW + dx + nt * 512 + 512],
                                start=(i == 0), stop=(i == len(dxs) - 1))
                            if i == len(dxs) - 1:
                                mm.then_inc(s_pe, 1)

        @block.scalar
        def _(act):
            for g in range(15):
                p, c = divmod(g, 5)
                act.wait_ge(s_pe, 4 * g + 2)
                act.copy(
                    out=DST[p][:, c * CW + 2:c * CW + 2 + 1024],
                    in_=ps[g % 2][:, 0:1024]).then_inc(s_cpy, 1)

        @block.vector
        def _(dve):
            dve.memset(wm[:, :], 0.0).then_inc(s_pad, 1)
            for c in range(NCHUNK):
                dve.memset(bass.AP(pong, c * CW,
                                   [[PINGW, 128], [2050, 2], [1, 2]]),
                           0.0).then_inc(s_pad, 1)
            dve.wait_ge(s_w2, 32)
            for g in range(15):
                p, c = divmod(g, 5)
                par = 4 * (g % 2)
                t1_ap = bass.AP(t1, par, [[8, 128], [2, 2], [1, 2]])
                t2_ap = bass.AP(t2, par, [[8, 128], [2, 2], [1, 2]])
                gc_ap = bass.AP(gcsb, p * 20 + c * 4,
                                [[60, 128], [2, 2], [1, 2]])
                sstrip = strip_ap(SRC[p], c * CW + 2, PINGW)
                dstrip = strip_ap(DST[p], c * CW + 2, PINGW)
                dve.wait_ge(s_pe, 4 * g + 4)
                dve.tensor_copy(
                    DST[p][:, c * CW + 2 + 1024:c * CW + 2 + 2048],
                    ps[g % 2][:, 1024:2048]).then_inc(s_cp2, 1)
                dve.wait_ge(s_cpy, g + 1)
                dve.wait_ge(s_cp2, g + 1)
                if g >= 2:
                    dve.wait_ge(s_str, g - 1)
                dve.tensor_sub(t1_ap, sstrip, dstrip).then_inc(s_t1, 1)
                dve.wait_ge(s_t1, 2 * g + 1)
                dve.tensor_mul(t2_ap, gc_ap, t1_ap).then_inc(s_t1, 1)
                dve.wait_ge(s_t1, 2 * g + 2)
                dve.tensor_sub(dstrip, sstrip, t2_ap).then_inc(s_str, 1)
    return nc


_LAST = None  # BassKernelResults of the most recent run (for test harness)


def kernel(X, y, kernel, N_norm):
    global _LAST
    from concourse.bass_utils import run_bass_kernel_spmd

    in_maps = _host_prep(X, y, kernel, N_norm)
    nc = _build_program()
    res = run_bass_kernel_spmd(nc, in_maps, list(range(BATCH * H_SHARDS)))
    _LAST = res

    yf = np.asarray(y)
    out = np.empty((BATCH, 1, M_IMG, N_IMG), np.float32)
    for core in range(BATCH * H_SHARDS):
        b, h = core // H_SHARDS, core % H_SHARDS
        out[b, 0, SH * h:SH * h + SH, :] = (
            yf[b, 0, SH * h:SH * h + SH, :]
            - res.results[core]["xo"].astype(np.float32))
    return out


# revision 32
# speedup vs baseline: 1.0261x; 1.0211x over previous
"""GuidedFilter (3-angle iterated boxfilter) on 8 trn2 NeuronCores.

Math: reference iterates  X <- X + (B_a(y) - B_a(X))/N_a  over 3 rotated-line
kernels B_a.  With the residual D = y - X this is  D <- D - B_a(D)/N_a,
one conv per angle, and  X_final = y - D_final  (computed on host).

Mapping: core (b, h) = (i//4, i%4) handles batch b, rows [512h, 512h+512).
Each core gets a 576-row slab (24-row shrink-halo per side, no cross-core
traffic), processed as 5 overlapping 128-row chunks (stride 112).  Per angle
and chunk the whole update Dnew = D - g_row*B'(D) (B' = B_a/s_a, identity
folded into the center-column band) is built on the TensorEngine as one
banded [128,128] bf16 matmul per nonzero kernel column (5 for +-10deg,
1 for 0deg), accumulated in PSUM.  ScalarE copies PSUM->SBUF bf16 per
512-column tile; DVE re-derives the 4 edge columns (where the N normalizer
differs per column) from src/dst:  Dnew = src - gc*(src - dst_wrong).
Chunk-overlap rows are synced by SBUF-to-SBUF DMA between passes.  D_final
goes out bf16; the host computes X = y - D in f32.  Dummy matmuls at start
keep the PE HAM clock-gate warm through the input-DMA phase.
"""

import numpy as np
import ml_dtypes

M_IMG = 2048
N_IMG = 2048
BATCH = 2
H_SHARDS = 4
SH = 512
SLAB = 576          # 512 + 2*32 shrink halo
CW = 2052           # bf16 master tile width (2 zero-pad cols each side)
NCHUNK = 5
CH_STEP = 112
KH = 17
PC = 8
PINGW = NCHUNK * CW

COLS = [(0, 1, 2, 3, 4), (2,), (0, 1, 2, 3, 4)]   # nonzero dx per angle
OFFS = [0, 15, 18]                                 # weight block base per angle
NWB = 33
OUT_ROWS = [(0, 32, 120), (88, 8, 120), (200, 8, 120), (312, 8, 120), (424, 8, 96)]
N_WARM = 40


def _host_prep(X, y, kern, N_norm):
    kern = np.asarray(kern, np.float64)[:, 0]        # (3,17,5)
    N = np.asarray(N_norm, np.float64)[:, 0]         # (3,2048,2048)
    D0 = (np.asarray(y) - np.asarray(X))[:, 0]       # (2,2048,2048) f32

    s = kern.sum(axis=(1, 2))
    grow_full = np.ones((3, M_IMG), np.float64)
    for a in range(3):
        grow_full[a] = s[a] / N[a, :, N_IMG // 2]

    BF16 = ml_dtypes.bfloat16

    in_maps = []
    for core in range(BATCH * H_SHARDS):
        b, h = core // H_SHARDS, core % H_SHARDS
        gs = SH * h - 32

        d0s = np.zeros((SLAB, CW), np.float32)
        r0, r1 = max(0, gs), min(M_IMG, gs + SLAB)
        d0s[r0 - gs:r1 - gs, 2:2 + N_IMG] = D0[b, r0:r1]

        # banded update matrices W = I - g*band, layout [k, idx*128 + m]
        wb = np.zeros((128, NWB * 128), np.float64)
        for a in range(3):
            for v in range(3):
                cv = {0: 0, 1: 1, 2: 4}[v]
                for m in range(8, 120):
                    go = gs + CH_STEP * cv + m
                    if not (0 <= go < M_IMG):
                        continue
                    g = grow_full[a][go]
                    for i, dx in enumerate(COLS[a]):
                        idx = OFFS[a] + v * len(COLS[a]) + i
                        for dy in range(KH):
                            if kern[a, dy, dx] == 0.0:
                                continue
                            wb[m - PC + dy, idx * 128 + m] -= \
                                g * kern[a, dy, dx] / s[a]
                        if dx == 2:
                            wb[m, idx * 128 + m] += 1.0

        # per-column strip factors gc = N(r,center)/N(r,c)
        gcs = np.ones((128, 3 * NCHUNK * 4), np.float64)
        scol = [0, 1, N_IMG - 2, N_IMG - 1]
        for a in range(3):
            for c in range(NCHUNK):
                g_glob = gs + CH_STEP * c + np.arange(128)
                ok = (g_glob >= 0) & (g_glob < M_IMG)
                gg = np.clip(g_glob, 0, M_IMG - 1)
                for j, cc in enumerate(scol):
                    v = N[a, gg, N_IMG // 2] / N[a, gg, cc]
                    gcs[:, a * 20 + c * 4 + j] = np.where(ok, v, 1.0)

        in_maps.append({
            "d0": d0s.astype(BF16),
            "wb": wb.astype(BF16),
            "gcs": gcs.astype(np.float32),
        })
    return in_maps


def _build_program():
    import concourse.bass as bass
    from concourse import mybir
    from contextlib import ExitStack

    f32 = mybir.dt.float32
    bf16 = mybir.dt.bfloat16
    nc = bass.Bass("TRN2", target_bir_lowering=False)

    d0 = nc.dram_tensor("d0", [SLAB, CW], bf16, kind="ExternalInput")
    wbd = nc.dram_tensor("wb", [128, NWB * 128], bf16, kind="ExternalInput")
    gcsd = nc.dram_tensor("gcs", [128, 60], f32, kind="ExternalInput")
    xo = nc.dram_tensor("xo", [SH, N_IMG], bf16, kind="ExternalOutput")

    ping = nc.alloc_sbuf_tensor("ping", [128, PINGW], bf16)
    pong = nc.alloc_sbuf_tensor("pong", [128, PINGW], bf16)
    wbs = nc.alloc_sbuf_tensor("wbs", [128, NWB * 128], bf16)
    gcsb = nc.alloc_sbuf_tensor("gcsb", [128, 60], f32)
    wm = nc.alloc_sbuf_tensor("wm", [128, 128], bf16)
    t1 = nc.alloc_sbuf_tensor("t1", [128, 8], f32)
    t2 = nc.alloc_sbuf_tensor("t2", [128, 8], f32)
    ps = [nc.alloc_psum_tensor(f"ps{i}", [128, N_IMG], f32) for i in range(2)]

    SRC = [ping, pong, ping]
    DST = [pong, ping, pong]
    # PE issue order: pass-1 chunks slot in behind their pass-0 producers
    SCHED = [(0, 0), (0, 1), (0, 2), (0, 3), (1, 0), (0, 4),
             (1, 1), (1, 2), (1, 3), (1, 4),
             (2, 0), (2, 1), (2, 2), (2, 3), (2, 4)]
    QIDX = {pc: q for q, pc in enumerate(SCHED)}

    def strip_ap(t, base, w):
        return bass.AP(t, base, [[w, 128], [2046, 2], [1, 2]])

    stack = ExitStack()
    with nc.Block() as block, \
         nc.semaphore("s_pe") as s_pe, nc.semaphore("s_cpy") as s_cpy, \
         nc.semaphore("s_str") as s_str, nc.semaphore("s_wa0") as s_wa0, \
         nc.semaphore("s_w2") as s_w2, nc.semaphore("s_out") as s_out, \
         nc.semaphore("s_pad") as s_pad, nc.semaphore("s_t1") as s_t1, \
         nc.semaphore("s_d34") as s_d34, nc.semaphore("s_cp2") as s_cp2, \
         nc.semaphore("s_wav0") as s_wav0:
        s_d0 = [stack.enter_context(nc.semaphore(f"s_d{c}")) for c in range(3)]
        s_h = [stack.enter_context(nc.semaphore(f"s_h{i}")) for i in range(8)]

        @block.sync
        def _(sp):
            sp.dma_start(out=ping[:, 0:CW],
                         in_=d0[0:128, :]).then_inc(s_d0[0], 16)
            sp.dma_start(out=wbs[:, 0:5 * 128],
                         in_=wbd[:, 0:5 * 128]).then_inc(s_wav0, 16)
            sp.dma_start(out=ping[:, CW:2 * CW],
                         in_=d0[CH_STEP:CH_STEP + 128, :]).then_inc(s_d0[1], 16)
            sp.dma_start(out=wbs[:, 5 * 128:OFFS[1] * 128],
                         in_=wbd[:, 5 * 128:OFFS[1] * 128]).then_inc(s_wa0, 16)
            sp.dma_start(out=ping[:, 2 * CW:3 * CW],
                         in_=d0[2 * CH_STEP:2 * CH_STEP + 128, :]
                         ).then_inc(s_d0[2], 16)
            sp.dma_start(out=gcsb[:, :], in_=gcsd[:, :]).then_inc(s_w2, 16)
            sp.dma_start(out=wbs[:, OFFS[1] * 128:NWB * 128],
                         in_=wbd[:, OFFS[1] * 128:NWB * 128]
                         ).then_inc(s_w2, 16)
            for c in (3, 4):
                sp.dma_start(out=ping[:, c * CW:(c + 1) * CW],
                             in_=d0[c * CH_STEP:c * CH_STEP + 128, :]
                             ).then_inc(s_d34, 16)
            # halo exchanges after pass 0 (on pong) and pass 1 (on ping)
            sp.wait_ge(s_pad, NCHUNK + 1)
            for t, tile in enumerate((pong, ping)):
                for bdy in range(NCHUNK - 1):
                    need = max(QIDX[(t, bdy)], QIDX[(t, bdy + 1)]) + 1
                    sp.wait_ge(s_cpy, need)
                    sp.wait_ge(s_cp2, need)
                    sp.wait_ge(s_str, need)
                    sp.dma_start(
                        out=tile[0:8, (bdy + 1) * CW:(bdy + 2) * CW],
                        in_=tile[112:120, bdy * CW:(bdy + 1) * CW]
                    ).then_inc(s_h[4 * t + bdy], 16)
                    sp.dma_start(
                        out=tile[120:128, bdy * CW:(bdy + 1) * CW],
                        in_=tile[8:16, (bdy + 1) * CW:(bdy + 2) * CW]
                    ).then_inc(s_h[4 * t + bdy], 16)
            for c in range(NCHUNK):
                o, p0, p1 = OUT_ROWS[c]
                sp.wait_ge(s_cpy, 11 + c)
                sp.wait_ge(s_cp2, 11 + c)
                sp.wait_ge(s_str, 11 + c)
                sp.dma_start(out=xo[o:o + (p1 - p0), :],
                             in_=pong[p0:p1, c * CW + 2:c * CW + 2 + N_IMG]
                             ).then_inc(s_out, 16)
            sp.wait_ge(s_out, 16 * NCHUNK)

        @block.tensor
        def _(pe):
            pe.wait_ge(s_pad, 1)
            for i in range(N_WARM):
                pe.matmul(ps[1][0:64, 0:64], lhsT=wm[:, 0:64],
                          rhs=wm[:, 64:128], start=True, stop=True)
            for q, (p, c) in enumerate(SCHED):
                dxs = COLS[p]
                v = {0: 0, 4: 2}.get(c, 1)
                if q >= 2:
                    pe.wait_ge(s_cpy, q - 1)
                if p == 0:
                    pe.wait_ge(s_wav0, 16)
                    if c > 0:
                        pe.wait_ge(s_wa0, 16)
                    if c < 3:
                        pe.wait_ge(s_d0[c], 16)
                    else:
                        pe.wait_ge(s_d34, 32)
                else:
                    pe.wait_ge(s_w2, 32)
                    hb = 4 * (p - 1)
                    if c > 0:
                        pe.wait_ge(s_h[hb + c - 1], 32)
                    pe.wait_ge(s_h[hb + min(c, 3)], 32)
                pst = ps[q % 2]
                for nt in range(4):
                    if nt == 2 and q >= 2:
                        pe.wait_ge(s_cp2, q - 1)
                    for i, dx in enumerate(dxs):
                        idx = OFFS[p] + v * len(dxs) + i
                        mm = pe.matmul(
                            pst[:, nt * 512:(nt + 1) * 512],
                            lhsT=wbs[:, idx * 128:(idx + 1) * 128],
                            rhs=SRC[p][:, c * CW + dx + nt * 512:
                                       c * CW + dx + nt * 512 + 512],
                            start=(i == 0), stop=(i == len(dxs) - 1))
                        if i == len(dxs) - 1:
                            mm.then_inc(s_pe, 1)

        @block.scalar
        def _(act):
            for q, (p, c) in enumerate(SCHED):
                act.wait_ge(s_pe, 4 * q + 2)
                act.copy(
                    out=DST[p][:, c * CW + 2:c * CW + 2 + 1024],
                    in_=ps[q % 2][:, 0:1024]).then_inc(s_cpy, 1)

        @block.vector
        def _(dve):
            dve.memset(wm[:, :], 0.0).then_inc(s_pad, 1)
            for c in range(NCHUNK):
                dve.memset(bass.AP(pong, c * CW,
                                   [[PINGW, 128], [2050, 2], [1, 2]]),
                           0.0).then_inc(s_pad, 1)
            dve.wait_ge(s_w2, 32)
            for q, (p, c) in enumerate(SCHED):
                par = 4 * (q % 2)
                t1_ap = bass.AP(t1, par, [[8, 128], [2, 2], [1, 2]])
                t2_ap = bass.AP(t2, par, [[8, 128], [2, 2], [1, 2]])
                gc_ap = bass.AP(gcsb, p * 20 + c * 4,
                                [[60, 128], [2, 2], [1, 2]])
                sstrip = strip_ap(SRC[p], c * CW + 2, PINGW)
                dstrip = strip_ap(DST[p], c * CW + 2, PINGW)
                dve.wait_ge(s_pe, 4 * q + 4)
                dve.tensor_copy(
                    DST[p][:, c * CW + 2 + 1024:c * CW + 2 + 2048],
                    ps[q % 2][:, 1024:2048]).then_inc(s_cp2, 1)
                dve.wait_ge(s_cpy, q + 1)
                dve.wait_ge(s_cp2, q + 1)
                if q >= 2:
                    dve.wait_ge(s_str, q - 1)
                dve.tensor_sub(t1_ap, sstrip, dstrip).then_inc(s_t1, 1)
                dve.wait_ge(s_t1, 2 * q + 1)
                dve.tensor_mul(t2_ap, gc_ap, t1_ap).then_inc(s_t1, 1)
                dve.wait_ge(s_t1, 2 * q + 2)
                dve.tensor_sub(dstrip, sstrip, t2_ap).then_inc(s_str, 1)
    return nc


_LAST = None  # BassKernelResults of the most recent run (for test harness)


def kernel(X, y, kernel, N_norm):
    global _LAST
    from concourse.bass_utils import run_bass_kernel_spmd

    in_maps = _host_prep(X, y, kernel, N_norm)
    nc = _build_program()
    res = run_bass_kernel_spmd(nc, in_maps, list(range(BATCH * H_SHARDS)))
    _LAST = res

    yf = np.asarray(y)
    out = np.empty((BATCH, 1, M_IMG, N_IMG), np.float32)
    for core in range(BATCH * H_SHARDS):
        b, h = core // H_SHARDS, core % H_SHARDS
        out[b, 0, SH * h:SH * h + SH, :] = (
            yf[b, 0, SH * h:SH * h + SH, :]
            - res.results[core]["xo"].astype(np.float32))
    return out
